# revision 8
# baseline (speedup 1.0000x reference)
"""Trainium2 Bass kernel for nn_CustomLlamaAttention (vq_codebook).

Sharding: tensor-parallel over heads. Core c owns q-heads [4c, 4c+4) and
kv-head c. Each core computes its heads' attention and a partial o_proj
(rows of Wo for its heads); the host sums the 8 partial outputs.

Host-side prep (sharding): slice the codebook indices per core and
reconstruct the per-core weight slices (bf16), cast hidden_states to bf16,
precompute RoPE tables and the causal tile mask.

Device kernel (per core, identical program, different inputs):
  - DMA-transpose hs (bf16) to get hsT [H, S] layout
  - QKV projections (bf16 matmuls, fp32 accum)
  - RoPE via partition-swapped copies + fused sign tables
  - attention with transposed-scores layout: sT[j,i] matmuls (row-tiled
    2 heads/instruction), exp on ACT, causal tile mask on DVE, attnV with a
    ones-column appended to V so the softmax denominator comes out of the
    same matmul, deferred normalization
  - partial o_proj (bf16) -> fp32 partial output
"""
import os
import sys

sys.path.insert(0, "/opt/trn_rl_repo")

import numpy as np
import ml_dtypes

import concourse.bacc as bacc
import concourse.mybir as mybir
import concourse.tile as tile
from concourse.bass_utils import run_bass_kernel_spmd
from concourse.masks import make_identity

H = 2048
NH = 32
NKV = 8
HD = 64
VDIM = 8
THETA = 10000.0
B, S = 2, 2048
NCORES = 8
P = 128
KC = H // P            # 16 k-chunks
HPC = NH // NCORES     # 4 q heads per core
NG = S // 512          # 4 col-groups of 512
BF16 = mybir.dt.bfloat16
F32 = mybir.dt.float32

_CACHE = {}


def _build_program():
    nc = bacc.Bacc("TRN2", target_bir_lowering=False, debug=False)

    hs_t = nc.dram_tensor("hs", (B, S, H), BF16, kind="ExternalInput").ap()
    wq_t = nc.dram_tensor("wq", (P, KC, 2 * P), BF16, kind="ExternalInput").ap()
    wkv_t = nc.dram_tensor("wkv", (P, KC, P), BF16, kind="ExternalInput").ap()
    wo_t = nc.dram_tensor("wo", (P, 2, H), BF16, kind="ExternalInput").ap()
    cosf_t = nc.dram_tensor("cosf", (P, S), F32, kind="ExternalInput").ap()
    sins_t = nc.dram_tensor("sins", (P, S), F32, kind="ExternalInput").ap()
    tri_t = nc.dram_tensor("tri", (P, P), BF16, kind="ExternalInput").ap()
    out_t = nc.dram_tensor("out", (B, S, H), F32, kind="ExternalOutput").ap()

    with tile.TileContext(nc) as tc:
        with tc.tile_pool(name="consts", bufs=1) as consts, \
             tc.tile_pool(name="weights", bufs=1) as wpool, \
             tc.tile_pool(name="hstp", bufs=16) as hstp, \
             tc.tile_pool(name="acts", bufs=1) as acts, \
             tc.tile_pool(name="swp", bufs=1) as swp, \
             tc.tile_pool(name="expp", bufs=6) as expp, \
             tc.tile_pool(name="nrm", bufs=2) as nrm, \
             tc.tile_pool(name="outp", bufs=3) as outp, \
             tc.tile_pool(name="ps_mm", bufs=1, space="PSUM") as ps_mm, \
             tc.tile_pool(name="ps_s", bufs=2, space="PSUM") as ps_s, \
             tc.tile_pool(name="ps_o", bufs=2, space="PSUM") as ps_o, \
             tc.tile_pool(name="ps_x", bufs=1, space="PSUM") as ps_x:

            # ---- constants / weights ----
            cosf = consts.tile([P, S], F32, tag="cosf")
            sins = consts.tile([P, S], F32, tag="sins")
            tri = consts.tile([P, P], BF16, tag="tri")
            ident = consts.tile([P, P], BF16, tag="ident")
            identf = consts.tile([P, P], F32, tag="identf")
            ones = consts.tile([P, HD], BF16, tag="ones")
            onesf = consts.tile([P, HD], F32, tag="onesf")
            nc.sync.dma_start(cosf[:], cosf_t[:])
            nc.sync.dma_start(sins[:], sins_t[:])
            nc.sync.dma_start(tri[:], tri_t[:])
            make_identity(nc, ident[:])
            make_identity(nc, identf[:])
            nc.vector.memset(ones[:], 1.0)
            nc.vector.memset(onesf[:], 1.0)

            wq = wpool.tile([P, KC, 2 * P], BF16, tag="wq")
            wkv = wpool.tile([P, KC, P], BF16, tag="wkv")
            wo = wpool.tile([P, 2, H], BF16, tag="wo")
            nc.sync.dma_start(wq[:], wq_t[:])
            nc.sync.dma_start(wkv[:], wkv_t[:])
            nc.sync.dma_start(wo[:], wo_t[:])

            for b in range(B):
                # ---- hsT via DMA transpose: [128, S] bf16 per k-chunk ----
                hsT = []
                for kc in range(KC):
                    t = hstp.tile([P, S], BF16, tag="hsT")
                    nc.sync.dma_start_transpose(t[:], hs_t[b, :, kc * P:(kc + 1) * P])
                    hsT.append(t)

                # ---- projections ----
                # qT pair tiles: [128, S] f32; rows = head (2*hp + m//64), d = m%64
                qT = [acts.tile([P, S], F32, tag=f"qT{hp}", name=f"qT{hp}") for hp in range(2)]
                kvT = acts.tile([P, S], F32, tag="kvT")   # rows 0:64 kT, 64:128 vT
                for hp in range(2):
                    for n in range(NG):
                        ps = ps_mm.tile([P, 512], F32, tag="mm")
                        for kc in range(KC):
                            nc.tensor.matmul(
                                ps[:], lhsT=wq[:, kc, hp * P:(hp + 1) * P],
                                rhs=hsT[kc][:, n * 512:(n + 1) * 512],
                                start=(kc == 0), stop=(kc == KC - 1))
                        nc.vector.tensor_copy(qT[hp][:, n * 512:(n + 1) * 512], ps[:])
                for n in range(NG):
                    ps = ps_mm.tile([P, 512], F32, tag="mm")
                    for kc in range(KC):
                        nc.tensor.matmul(
                            ps[:], lhsT=wkv[:, kc, :],
                            rhs=hsT[kc][:, n * 512:(n + 1) * 512],
                            start=(kc == 0), stop=(kc == KC - 1))
                    nc.vector.tensor_copy(kvT[:, n * 512:(n + 1) * 512], ps[:])

                # ---- RoPE (in-place) on qT pairs and kT (kvT[0:64]) ----
                def rope(tile_ap, nhalf):
                    # tile_ap: [64*nhalf... ] layout: blocks of 64 per head
                    sw = swp.tile([P, S], F32, tag="swap")
                    for blk in range(nhalf):
                        b0 = blk * 64
                        nc.sync.dma_start(sw[b0:b0 + 32, :], tile_ap[b0 + 32:b0 + 64, :])
                        nc.sync.dma_start(sw[b0 + 32:b0 + 64, :], tile_ap[b0:b0 + 32, :])
                    rows = nhalf * 64
                    nc.vector.tensor_mul(sw[0:rows], sw[0:rows], sins[0:rows])
                    nc.vector.tensor_mul(tile_ap[0:rows], tile_ap[0:rows], cosf[0:rows])
                    nc.vector.tensor_add(tile_ap[0:rows], tile_ap[0:rows], sw[0:rows])

                rope(qT[0][:], 2)
                rope(qT[1][:], 2)
                rope(kvT[:], 1)

                # ---- bf16 casts for attention operands ----
                qTb = [acts.tile([P, S], BF16, tag=f"qTb{hp}", name=f"qTb{hp}")
                       for hp in range(2)]
                nc.vector.tensor_copy(qTb[0][:], qT[0][:])
                nc.vector.tensor_copy(qTb[1][:], qT[1][:])
                # kkTb: roped kT cast + duplicated to both halves (SWDGE casts)
                kkTb = acts.tile([P, S], BF16, tag="kkTb")
                nc.gpsimd.dma_start(kkTb[0:64, :], kvT[0:64, :])
                nc.gpsimd.dma_start(kkTb[64:128, :], kvT[0:64, :])

                # ---- v1: [128, 16, 65] bf16; v[jb] rows + ones column ----
                v1 = acts.tile([P, KC, 80], BF16, tag="v1")
                nc.vector.memset(v1[:, :, HD:HD + 1], 1.0)
                for jb in range(KC):
                    pst = ps_x.tile([P, 512], F32, tag="x")
                    nc.tensor.transpose(pst[:, 0:HD], kvT[64:128, jb * P:(jb + 1) * P],
                                        identf[64:128, 64:128])
                    nc.vector.tensor_copy(v1[:, jb, 0:HD], pst[:, 0:HD])

                # ---- attention per head pair ----
                # chunk tiles for o_proj lhsT: [128, S] bf16 (heads 2hp, 2hp+1)
                chunk = [acts.tile([P, S], BF16, tag=f"chunk{hp}", name=f"chunk{hp}") for hp in range(2)]
                for hp in range(2):
                    for g in range(NG):
                        po = [ps_o.tile([P, 512], F32, tag=f"o{x}", name=f"po{x}") for x in range(2)]
                        njb = 4 * g + 4
                        for jb in range(njb):
                            t = jb - 4 * g
                            ex = []
                            for x in range(2):
                                pss = ps_s.tile([P, 512], F32, tag="s")
                                nc.tensor.matmul(
                                    pss[:],
                                    lhsT=kkTb[64 * x:64 * x + 64, jb * P:(jb + 1) * P],
                                    rhs=qTb[hp][64 * x:64 * x + 64, g * 512:(g + 1) * 512],
                                    start=True, stop=True, tile_position=(64 * x, 0))
                                e = expp.tile([P, 512], BF16, tag="e")
                                off = max(t, 0) * P
                                if off > 0:
                                    nc.vector.memset(e[:, 0:off], 0.0)
                                nc.scalar.activation(e[:, off:], pss[:, off:],
                                                     mybir.ActivationFunctionType.Exp,
                                                     scale=0.125)
                                if 0 <= t:
                                    nc.vector.tensor_mul(
                                        e[:, t * P:(t + 1) * P],
                                        e[:, t * P:(t + 1) * P], tri[:])
                                ex.append(e)
                            for x in range(2):
                                nc.tensor.matmul(
                                    po[x][0:HD + 1, :], lhsT=v1[:, jb, 0:HD + 1], rhs=ex[x][:],
                                    start=(jb == 0), stop=(jb == njb - 1))
                        # normalize + place into chunk
                        for x in range(2):
                            rec = nrm.tile([P, 512], F32, tag="rec")
                            nc.vector.reciprocal(rec[HD:HD + 1, :], po[x][HD:HD + 1, :])
                            pbc = ps_x.tile([P, 512], F32, tag="x")
                            nc.tensor.matmul(pbc[0:HD, :], lhsT=onesf[HD:HD + 1, 0:HD],
                                             rhs=rec[HD:HD + 1, :], start=True, stop=True)
                            bc = nrm.tile([P, 512], F32, tag="bc")
                            nc.vector.tensor_copy(bc[0:HD, :], pbc[0:HD, :])
                            if x == 0:
                                nc.vector.tensor_mul(
                                    chunk[hp][0:HD, g * 512:(g + 1) * 512],
                                    po[x][0:HD, :], bc[0:HD, :])
                            else:
                                tmp = nrm.tile([P, 512], BF16, tag="tmp")
                                nc.vector.tensor_mul(tmp[0:HD, :], po[x][0:HD, :],
                                                     bc[0:HD, :])
                                psh = ps_x.tile([P, 512], F32, tag="x")
                                nc.tensor.matmul(psh[64:128, :],
                                                 lhsT=ident[0:64, 0:64],
                                                 rhs=tmp[0:HD, :],
                                                 start=True, stop=True,
                                                 tile_position=(0, 64))
                                nc.vector.tensor_copy(
                                    chunk[hp][64:128, g * 512:(g + 1) * 512],
                                    psh[64:128, :])

                # ---- o_proj partial: out[b] = chunks.T @ wo ----
                for m in range(KC):
                    for n in range(NG):
                        ps = ps_mm.tile([P, 512], F32, tag="mm")
                        for ct in range(2):
                            nc.tensor.matmul(
                                ps[:], lhsT=chunk[ct][:, m * P:(m + 1) * P],
                                rhs=wo[:, ct, n * 512:(n + 1) * 512],
                                start=(ct == 0), stop=(ct == 1))
                        ot = outp.tile([P, 512], F32, tag="ot")
                        nc.vector.tensor_copy(ot[:], ps[:])
                        nc.sync.dma_start(
                            out_t[b, m * P:(m + 1) * P, n * 512:(n + 1) * 512], ot[:])

    nc.compile()
    return nc


def _host_prep(hidden_states, vector_bank, q_idx, k_idx, v_idx, o_idx):
    hs_bf16 = np.ascontiguousarray(hidden_states).astype(ml_dtypes.bfloat16)
    bank = np.asarray(vector_bank)

    # RoPE tables in qT layout ([d, s], two 64-blocks stacked)
    inv_freq = 1.0 / (THETA ** (np.arange(0, HD, 2, dtype=np.float32) / HD))  # [32]
    pos = np.arange(S, dtype=np.float32)
    ang = pos[None, :] * inv_freq[:, None]          # [32, S]
    cos = np.cos(ang).astype(np.float32)
    sin = np.sin(ang).astype(np.float32)
    cos64 = np.concatenate([cos, cos], axis=0)       # [64, S]
    sin_signed = np.concatenate([-sin, sin], axis=0)  # [64, S]
    cosf = np.tile(cos64, (2, 1))                    # [128, S]
    sins = np.tile(sin_signed, (2, 1))
    tri = np.triu(np.ones((P, P), dtype=np.float32)).astype(ml_dtypes.bfloat16)

    q_grid = np.asarray(q_idx).reshape(H, 256)
    k_grid = np.asarray(k_idx).reshape(H, 64)
    v_grid = np.asarray(v_idx).reshape(H, 64)
    o_grid = np.asarray(o_idx).reshape(H, 256)

    in_maps = []
    for c in range(NCORES):
        # Wq slice: cols [c*256, (c+1)*256) -> [128, KC, 256] bf16
        wq = bank[q_grid[:, c * 32:(c + 1) * 32].ravel()].reshape(H, 256)
        wq = wq.reshape(KC, P, 256).transpose(1, 0, 2).astype(ml_dtypes.bfloat16)
        # Wk|Wv slice: cols [c*64, (c+1)*64) each -> [128, KC, 128]
        wk = bank[k_grid[:, c * 8:(c + 1) * 8].ravel()].reshape(H, 64)
        wv = bank[v_grid[:, c * 8:(c + 1) * 8].ravel()].reshape(H, 64)
        wkv = np.concatenate([wk, wv], axis=1)
        wkv = wkv.reshape(KC, P, P).transpose(1, 0, 2).astype(ml_dtypes.bfloat16)
        # Wo rows [c*256, (c+1)*256) -> [128, 2, 2048]
        wos = bank[o_grid[c * 256:(c + 1) * 256, :].ravel()].reshape(256, H)
        wos = wos.reshape(2, P, H).transpose(1, 0, 2).astype(ml_dtypes.bfloat16)
        in_maps.append({
            "hs": hs_bf16,
            "wq": np.ascontiguousarray(wq),
            "wkv": np.ascontiguousarray(wkv),
            "wo": np.ascontiguousarray(wos),
            "cosf": cosf,
            "sins": sins,
            "tri": tri,
        })
    return in_maps


def kernel(hidden_states, vector_bank, q_idx, k_idx, v_idx, o_idx,
           trace=False, trace_cores=None):
    if "nc" not in _CACHE:
        _CACHE["nc"] = _build_program()
    nc = _CACHE["nc"]
    in_maps = _host_prep(hidden_states, vector_bank, q_idx, k_idx, v_idx, o_idx)
    res = run_bass_kernel_spmd(nc, in_maps, core_ids=list(range(NCORES)),
                               trace=trace, trace_cores=trace_cores)
    _CACHE["last_result"] = res
    out = np.zeros((B, S, H), dtype=np.float64)
    for r in res.results:
        out += r["out"].astype(np.float64)
    return out.astype(np.float32)


if __name__ == "__main__":
    rng = np.random.default_rng(0)
    inputs = {
        "hidden_states": rng.standard_normal((B, S, H), dtype=np.float32) * 0.02,
        "vector_bank": rng.standard_normal((H * H // VDIM, VDIM), dtype=np.float32) * 0.02,
        "q_idx": rng.integers(0, H * H // VDIM, (H * H // VDIM,)).astype(np.int32),
        "k_idx": rng.integers(0, H * 512 // VDIM, (H * 512 // VDIM,)).astype(np.int32),
        "v_idx": rng.integers(0, H * 512 // VDIM, (H * 512 // VDIM,)).astype(np.int32),
        "o_idx": rng.integers(0, H * H // VDIM, (H * H // VDIM,)).astype(np.int32),
    }
    out = kernel(**inputs)
    print("kernel ran, out shape", out.shape, "std", out.std())


# revision 9
# speedup vs baseline: 1.2104x; 1.2104x over previous
"""Trainium2 Bass kernel for nn_CustomLlamaAttention (vq_codebook).

Sharding: tensor-parallel over heads. Core c owns q-heads [4c, 4c+4) and
kv-head c. Each core computes its heads' attention and a partial o_proj
(rows of Wo for its heads); the host sums the 8 partial outputs.

Host-side prep (sharding): slice the codebook indices per core and
reconstruct the per-core weight slices (bf16), cast hidden_states to bf16,
precompute RoPE tables and the causal tile mask.

Device kernel (per core, identical program, different inputs):
  - DMA-transpose hs (bf16) to get hsT [H, S] layout
  - QKV projections (bf16 matmuls, fp32 accum)
  - RoPE via partition-swapped copies + fused sign tables
  - attention with transposed-scores layout: sT[j,i] matmuls (row-tiled
    2 heads/instruction), exp on ACT, causal tile mask on DVE, attnV with a
    ones-column appended to V so the softmax denominator comes out of the
    same matmul, deferred normalization
  - partial o_proj (bf16) -> fp32 partial output
"""
import os
import sys

sys.path.insert(0, "/opt/trn_rl_repo")

import numpy as np
import ml_dtypes

import concourse.bacc as bacc
import concourse.mybir as mybir
import concourse.tile as tile
from concourse.bass_utils import run_bass_kernel_spmd
from concourse.masks import make_identity

H = 2048
NH = 32
NKV = 8
HD = 64
VDIM = 8
THETA = 10000.0
B, S = 2, 2048
NCORES = 8
P = 128
KC = H // P            # 16 k-chunks
HPC = NH // NCORES     # 4 q heads per core
NG = S // 512          # 4 col-groups of 512
BF16 = mybir.dt.bfloat16
F32 = mybir.dt.float32

_CACHE = {}


def _build_program():
    nc = bacc.Bacc("TRN2", target_bir_lowering=False, debug=False)

    hs_t = nc.dram_tensor("hs", (B, S, H), BF16, kind="ExternalInput").ap()
    wq_t = nc.dram_tensor("wq", (P, KC, 2 * P), BF16, kind="ExternalInput").ap()
    wkv_t = nc.dram_tensor("wkv", (P, KC, P), BF16, kind="ExternalInput").ap()
    wo_t = nc.dram_tensor("wo", (P, 2, H), BF16, kind="ExternalInput").ap()
    cosf_t = nc.dram_tensor("cosf", (P, S), F32, kind="ExternalInput").ap()
    sins_t = nc.dram_tensor("sins", (P, S), F32, kind="ExternalInput").ap()
    tri_t = nc.dram_tensor("tri", (P, P), BF16, kind="ExternalInput").ap()
    out_t = nc.dram_tensor("out", (B, S, H), F32, kind="ExternalOutput").ap()

    with tile.TileContext(nc) as tc:
        with tc.tile_pool(name="consts", bufs=1) as consts, \
             tc.tile_pool(name="weights", bufs=1) as wpool, \
             tc.tile_pool(name="hstp", bufs=16) as hstp, \
             tc.tile_pool(name="acts", bufs=1) as acts, \
             tc.tile_pool(name="swp", bufs=1) as swp, \
             tc.tile_pool(name="expp", bufs=6) as expp, \
             tc.tile_pool(name="nrm", bufs=2) as nrm, \
             tc.tile_pool(name="outp", bufs=3) as outp, \
             tc.tile_pool(name="ps_mm", bufs=2, space="PSUM") as ps_mm, \
             tc.tile_pool(name="ps_s", bufs=3, space="PSUM") as ps_s, \
             tc.tile_pool(name="ps_o", bufs=1, space="PSUM") as ps_o, \
             tc.tile_pool(name="ps_x", bufs=1, space="PSUM") as ps_x:

            # ---- constants / weights ----
            cosf = consts.tile([P, S], F32, tag="cosf")
            sins = consts.tile([P, S], F32, tag="sins")
            tri = consts.tile([P, P], BF16, tag="tri")
            ident = consts.tile([P, P], BF16, tag="ident")
            identf = consts.tile([P, P], F32, tag="identf")
            ones = consts.tile([P, HD], BF16, tag="ones")
            onesf = consts.tile([P, HD], F32, tag="onesf")
            nc.sync.dma_start(cosf[:], cosf_t[:])
            nc.sync.dma_start(sins[:], sins_t[:])
            nc.sync.dma_start(tri[:], tri_t[:])
            make_identity(nc, ident[:])
            make_identity(nc, identf[:])
            nc.vector.memset(ones[:], 1.0)
            nc.vector.memset(onesf[:], 1.0)

            wq = wpool.tile([P, KC, 2 * P], BF16, tag="wq")
            wkv = wpool.tile([P, KC, P], BF16, tag="wkv")
            wo = wpool.tile([P, 2, H], BF16, tag="wo")
            nc.sync.dma_start(wq[:], wq_t[:])
            nc.sync.dma_start(wkv[:], wkv_t[:])
            nc.sync.dma_start(wo[:], wo_t[:])

            for b in range(B):
                # ---- hsT via DMA transpose: [128, S] bf16 per k-chunk ----
                hsT = []
                for kc in range(KC):
                    t = hstp.tile([P, S], BF16, tag="hsT")
                    nc.sync.dma_start_transpose(t[:], hs_t[b, :, kc * P:(kc + 1) * P])
                    hsT.append(t)

                # ---- projections ----
                # qT pair tiles: [128, S] f32; rows = head (2*hp + m//64), d = m%64
                qT = [acts.tile([P, S], F32, tag=f"qT{hp}", name=f"qT{hp}") for hp in range(2)]
                kvT = acts.tile([P, S], F32, tag="kvT")   # rows 0:64 kT, 64:128 vT
                for hp in range(2):
                    for n in range(NG):
                        ps = ps_mm.tile([P, 512], F32, tag="mm")
                        for kc in range(KC):
                            nc.tensor.matmul(
                                ps[:], lhsT=wq[:, kc, hp * P:(hp + 1) * P],
                                rhs=hsT[kc][:, n * 512:(n + 1) * 512],
                                start=(kc == 0), stop=(kc == KC - 1))
                        nc.vector.tensor_copy(qT[hp][:, n * 512:(n + 1) * 512], ps[:])
                for n in range(NG):
                    ps = ps_mm.tile([P, 512], F32, tag="mm")
                    for kc in range(KC):
                        nc.tensor.matmul(
                            ps[:], lhsT=wkv[:, kc, :],
                            rhs=hsT[kc][:, n * 512:(n + 1) * 512],
                            start=(kc == 0), stop=(kc == KC - 1))
                    nc.vector.tensor_copy(kvT[:, n * 512:(n + 1) * 512], ps[:])

                # ---- RoPE (in-place) on qT pairs and kT (kvT[0:64]) ----
                def rope(tile_ap, nhalf):
                    # tile_ap: [64*nhalf... ] layout: blocks of 64 per head
                    sw = swp.tile([P, S], F32, tag="swap")
                    for blk in range(nhalf):
                        b0 = blk * 64
                        nc.sync.dma_start(sw[b0:b0 + 32, :], tile_ap[b0 + 32:b0 + 64, :])
                        nc.sync.dma_start(sw[b0 + 32:b0 + 64, :], tile_ap[b0:b0 + 32, :])
                    rows = nhalf * 64
                    nc.vector.tensor_mul(sw[0:rows], sw[0:rows], sins[0:rows])
                    nc.vector.tensor_mul(tile_ap[0:rows], tile_ap[0:rows], cosf[0:rows])
                    nc.vector.tensor_add(tile_ap[0:rows], tile_ap[0:rows], sw[0:rows])

                rope(qT[0][:], 2)
                rope(qT[1][:], 2)
                rope(kvT[:], 1)

                # ---- bf16 casts for attention operands ----
                qTb = [acts.tile([P, S], BF16, tag=f"qTb{hp}", name=f"qTb{hp}")
                       for hp in range(2)]
                nc.vector.tensor_copy(qTb[0][:], qT[0][:])
                nc.vector.tensor_copy(qTb[1][:], qT[1][:])
                # kkTb: roped kT cast + duplicated to both halves (SWDGE casts)
                kkTb = acts.tile([P, S], BF16, tag="kkTb")
                nc.gpsimd.dma_start(kkTb[0:64, :], kvT[0:64, :])
                nc.gpsimd.dma_start(kkTb[64:128, :], kvT[0:64, :])

                # ---- v1: [128, 16, 65] bf16; v[jb] rows + ones column ----
                v1 = acts.tile([P, KC, 80], BF16, tag="v1")
                nc.vector.memset(v1[:, :, HD:HD + 1], 1.0)
                for jb in range(KC):
                    pst = ps_x.tile([P, 512], F32, tag="x")
                    nc.tensor.transpose(pst[:, 0:HD], kvT[64:128, jb * P:(jb + 1) * P],
                                        identf[64:128, 64:128])
                    nc.vector.tensor_copy(v1[:, jb, 0:HD], pst[:, 0:HD])

                # ---- attention per head pair ----
                # chunk tiles for o_proj lhsT: [128, S] bf16 (heads 2hp, 2hp+1)
                chunk = [acts.tile([P, S], BF16, tag=f"chunk{hp}", name=f"chunk{hp}") for hp in range(2)]
                for hp in range(2):
                    for g in range(NG):
                        po = [ps_o.tile([P, 512], F32, tag=f"o{x}", name=f"po{x}") for x in range(2)]
                        njb = 4 * g + 4
                        for jb in range(njb):
                            t = jb - 4 * g
                            ex = []
                            for x in range(2):
                                pss = ps_s.tile([P, 512], F32, tag="s")
                                nc.tensor.matmul(
                                    pss[:],
                                    lhsT=kkTb[64 * x:64 * x + 64, jb * P:(jb + 1) * P],
                                    rhs=qTb[hp][64 * x:64 * x + 64, g * 512:(g + 1) * 512],
                                    start=True, stop=True, tile_position=(64 * x, 0))
                                e = expp.tile([P, 512], BF16, tag="e")
                                off = max(t, 0) * P
                                if off > 0:
                                    nc.vector.memset(e[:, 0:off], 0.0)
                                nc.scalar.activation(e[:, off:], pss[:, off:],
                                                     mybir.ActivationFunctionType.Exp,
                                                     scale=0.125)
                                if 0 <= t:
                                    nc.vector.tensor_mul(
                                        e[:, t * P:(t + 1) * P],
                                        e[:, t * P:(t + 1) * P], tri[:])
                                ex.append(e)
                            for x in range(2):
                                nc.tensor.matmul(
                                    po[x][0:HD + 1, :], lhsT=v1[:, jb, 0:HD + 1], rhs=ex[x][:],
                                    start=(jb == 0), stop=(jb == njb - 1))
                        # normalize + place into chunk
                        for x in range(2):
                            rec = nrm.tile([P, 512], F32, tag="rec")
                            nc.vector.reciprocal(rec[HD:HD + 1, :], po[x][HD:HD + 1, :])
                            pbc = ps_x.tile([P, 512], F32, tag="x")
                            nc.tensor.matmul(pbc[0:HD, :], lhsT=onesf[HD:HD + 1, 0:HD],
                                             rhs=rec[HD:HD + 1, :], start=True, stop=True)
                            bc = nrm.tile([P, 512], F32, tag="bc")
                            nc.vector.tensor_copy(bc[0:HD, :], pbc[0:HD, :])
                            if x == 0:
                                nc.vector.tensor_mul(
                                    chunk[hp][0:HD, g * 512:(g + 1) * 512],
                                    po[x][0:HD, :], bc[0:HD, :])
                            else:
                                tmp = nrm.tile([P, 512], BF16, tag="tmp")
                                nc.vector.tensor_mul(tmp[0:HD, :], po[x][0:HD, :],
                                                     bc[0:HD, :])
                                psh = ps_x.tile([P, 512], F32, tag="x")
                                nc.tensor.matmul(psh[64:128, :],
                                                 lhsT=ident[0:64, 0:64],
                                                 rhs=tmp[0:HD, :],
                                                 start=True, stop=True,
                                                 tile_position=(0, 64))
                                nc.vector.tensor_copy(
                                    chunk[hp][64:128, g * 512:(g + 1) * 512],
                                    psh[64:128, :])

                # ---- o_proj partial: out[b] = chunks.T @ wo ----
                for m in range(KC):
                    for n in range(NG):
                        ps = ps_mm.tile([P, 512], F32, tag="mm")
                        for ct in range(2):
                            nc.tensor.matmul(
                                ps[:], lhsT=chunk[ct][:, m * P:(m + 1) * P],
                                rhs=wo[:, ct, n * 512:(n + 1) * 512],
                                start=(ct == 0), stop=(ct == 1))
                        ot = outp.tile([P, 512], F32, tag="ot")
                        nc.vector.tensor_copy(ot[:], ps[:])
                        nc.sync.dma_start(
                            out_t[b, m * P:(m + 1) * P, n * 512:(n + 1) * 512], ot[:])

    nc.compile()
    return nc


def _host_prep(hidden_states, vector_bank, q_idx, k_idx, v_idx, o_idx):
    hs_bf16 = np.ascontiguousarray(hidden_states).astype(ml_dtypes.bfloat16)
    bank = np.asarray(vector_bank)

    # RoPE tables in qT layout ([d, s], two 64-blocks stacked)
    inv_freq = 1.0 / (THETA ** (np.arange(0, HD, 2, dtype=np.float32) / HD))  # [32]
    pos = np.arange(S, dtype=np.float32)
    ang = pos[None, :] * inv_freq[:, None]          # [32, S]
    cos = np.cos(ang).astype(np.float32)
    sin = np.sin(ang).astype(np.float32)
    cos64 = np.concatenate([cos, cos], axis=0)       # [64, S]
    sin_signed = np.concatenate([-sin, sin], axis=0)  # [64, S]
    cosf = np.tile(cos64, (2, 1))                    # [128, S]
    sins = np.tile(sin_signed, (2, 1))
    tri = np.triu(np.ones((P, P), dtype=np.float32)).astype(ml_dtypes.bfloat16)

    q_grid = np.asarray(q_idx).reshape(H, 256)
    k_grid = np.asarray(k_idx).reshape(H, 64)
    v_grid = np.asarray(v_idx).reshape(H, 64)
    o_grid = np.asarray(o_idx).reshape(H, 256)

    in_maps = []
    for c in range(NCORES):
        # Wq slice: cols [c*256, (c+1)*256) -> [128, KC, 256] bf16
        wq = bank[q_grid[:, c * 32:(c + 1) * 32].ravel()].reshape(H, 256)
        wq = wq.reshape(KC, P, 256).transpose(1, 0, 2).astype(ml_dtypes.bfloat16)
        # Wk|Wv slice: cols [c*64, (c+1)*64) each -> [128, KC, 128]
        wk = bank[k_grid[:, c * 8:(c + 1) * 8].ravel()].reshape(H, 64)
        wv = bank[v_grid[:, c * 8:(c + 1) * 8].ravel()].reshape(H, 64)
        wkv = np.concatenate([wk, wv], axis=1)
        wkv = wkv.reshape(KC, P, P).transpose(1, 0, 2).astype(ml_dtypes.bfloat16)
        # Wo rows [c*256, (c+1)*256) -> [128, 2, 2048]
        wos = bank[o_grid[c * 256:(c + 1) * 256, :].ravel()].reshape(256, H)
        wos = wos.reshape(2, P, H).transpose(1, 0, 2).astype(ml_dtypes.bfloat16)
        in_maps.append({
            "hs": hs_bf16,
            "wq": np.ascontiguousarray(wq),
            "wkv": np.ascontiguousarray(wkv),
            "wo": np.ascontiguousarray(wos),
            "cosf": cosf,
            "sins": sins,
            "tri": tri,
        })
    return in_maps


def kernel(hidden_states, vector_bank, q_idx, k_idx, v_idx, o_idx,
           trace=False, trace_cores=None):
    if "nc" not in _CACHE:
        _CACHE["nc"] = _build_program()
    nc = _CACHE["nc"]
    in_maps = _host_prep(hidden_states, vector_bank, q_idx, k_idx, v_idx, o_idx)
    res = run_bass_kernel_spmd(nc, in_maps, core_ids=list(range(NCORES)),
                               trace=trace, trace_cores=trace_cores)
    _CACHE["last_result"] = res
    out = np.zeros((B, S, H), dtype=np.float64)
    for r in res.results:
        out += r["out"].astype(np.float64)
    return out.astype(np.float32)


if __name__ == "__main__":
    rng = np.random.default_rng(0)
    inputs = {
        "hidden_states": rng.standard_normal((B, S, H), dtype=np.float32) * 0.02,
        "vector_bank": rng.standard_normal((H * H // VDIM, VDIM), dtype=np.float32) * 0.02,
        "q_idx": rng.integers(0, H * H // VDIM, (H * H // VDIM,)).astype(np.int32),
        "k_idx": rng.integers(0, H * 512 // VDIM, (H * 512 // VDIM,)).astype(np.int32),
        "v_idx": rng.integers(0, H * 512 // VDIM, (H * 512 // VDIM,)).astype(np.int32),
        "o_idx": rng.integers(0, H * H // VDIM, (H * H // VDIM,)).astype(np.int32),
    }
    out = kernel(**inputs)
    print("kernel ran, out shape", out.shape, "std", out.std())
